# revision 6
# baseline (speedup 1.0000x reference)
"""Causal self-attention (B=2, T=2048, C=1024, H=16) on 8 TRN2 NeuronCores.

Sharding: 8 cores = 2 batches x 4 head-groups (4 heads each).
Each core computes qkv projection for its heads, attention, and a partial
output projection (its rows of w_proj); the host sums the 4 partials per
batch and adds b_proj.

v3 (from v2's globally software-pipelined single stream):
  - xt SBUF layout is chunk-contiguous [tci][cb][t'] matching the host
    pack, so each chunk DMA is one 8KB span per partition (the v2 layout
    forced 1KB-granular descriptors; ~12us of descriptor-gen per chunk).
  - Input DMAs split across sync (xt) + gpsimd (weights) queues,
    first-needed-first, finer granularity so matmuls chase arrival.
  - Scalar engine runs (almost) only exp: the qb Identity-activation is
    gone - rope swap-muls read pq straight from PSUM via STT with the
    bias folded into the scalar operand.
  - Diagonal tiles: one exp instr (2-level AP over both heads' live
    ranges) and one affine_select per tile instead of two.
  - Softmax normalize: partition_broadcast of the denominator row
    (straight from PSUM), reciprocal on the broadcast [64,512] tile, one
    multiply; emission deferred into the NEXT pair's stream so it can't
    head-of-line-block the DVE queue (v2 lost ~6us to that at the
    attn(0)->attn(1) boundary).
  - Filler rebalance: outproj(0)/(1) pulled up into attn(1)/(2); tail
    outproj PSUM evictions on the scalar engine (idle at the tail).
"""

import sys
import os

for _p in ("/opt/trn_rl_repo", "/root/.axon_site/_ro/trn_rl_repo"):
    if os.path.isdir(_p) and _p not in sys.path:
        sys.path.insert(0, _p)

import numpy as np
import concourse.bass as bass
import concourse.mybir as mybir
import concourse.tile as tile
from concourse import bacc
from concourse.bass_utils import run_bass_kernel_spmd

B, T, C, H = 2, 2048, 1024, 16
HS = C // H          # 64
HALF = HS // 2       # 32
NCORES = 8
NH = 4               # heads per core
TCH = 512            # t-chunk for projections / i-chunk for attention
NCH = T // TCH       # 4 chunks
CB = C // 128        # 8 contraction blocks
NTB = T // 128       # 16 t/j blocks
F32 = mybir.dt.float32
BF16 = mybir.dt.bfloat16
MMD = BF16
AF = mybir.ActivationFunctionType
ALU = mybir.AluOpType

TRIM = os.environ.get("KTRIM", "1") == "1"       # causal QK/exp/AV trims
AV_DEPTH = int(os.environ.get("KAVDEPTH", "2"))  # AV pipeline lookahead
# qb eviction on scalar: keeping it OFF makes the rope swap-muls read pq
# from PSUM via STT, which measured ~2x slower per op AND slowed exp by
# ~20% through PSUM port contention. Keep ON.
QB_ACT = os.environ.get("KQBACT", "1") == "1"

_CACHED = {}


def _build_nc():
    nc = bacc.Bacc("TRN2", target_bir_lowering=False, debug=False)

    # dram layouts are partition-major with long contiguous rows so each
    # DMA descriptor moves a big span; host prepacks accordingly
    xt = nc.dram_tensor("xt", [128, NCH * CB * TCH], MMD, kind="ExternalInput").ap()
    wqk = nc.dram_tensor("wqk", [128, CB * 512], MMD, kind="ExternalInput").ap()
    wv = nc.dram_tensor("wv", [128, CB * 256], MMD, kind="ExternalInput").ap()
    wproj = nc.dram_tensor("wproj", [128, 2 * C], MMD, kind="ExternalInput").ap()
    bqk = nc.dram_tensor("bqk", [128, 4], F32, kind="ExternalInput").ap()
    cosrep = nc.dram_tensor("cosrep", [128, T], MMD, kind="ExternalInput").ap()
    sinsw = nc.dram_tensor("sinsw", [128, T], MMD, kind="ExternalInput").ap()
    yout = nc.dram_tensor("yout", [T, C], MMD, kind="ExternalOutput").ap()

    with tile.TileContext(nc) as tc:
        with (
            tc.tile_pool(name="const", bufs=1) as const,
            tc.tile_pool(name="persist", bufs=1) as persist,
            tc.tile_pool(name="work", bufs=2) as work,
            tc.tile_pool(name="attnp", bufs=6) as attnp,
            tc.tile_pool(name="pst", bufs=2, space="PSUM") as pst,
            tc.tile_pool(name="ppj", bufs=2, space="PSUM") as ppj,
            tc.tile_pool(name="pctx", bufs=1, space="PSUM") as pctx,
        ):
            # ---- DMA: xt on sync, weights on gpsimd, first-needed first.
            # xt_sb columns are [tci][cb][t'] = exactly the dram row layout,
            # so every transfer is long-contiguous on both sides.
            wqk_sb = const.tile([128, CB * 512], MMD)
            xt_sb = const.tile([128, NCH * CB * TCH], MMD)  # all chunks resident

            def xt_slice(tci, cb, t0=0, t1=TCH):
                base = tci * CB * TCH + cb * TCH
                return xt_sb[:, base + t0 : base + t1]

            xt_dst = xt_sb.rearrange("p (tci cb t) -> p tci cb t", tci=NCH, cb=CB)
            xt_src = xt.rearrange("p (tci cb t) -> p tci cb t", tci=NCH, cb=CB)
            wqk_dst = wqk_sb.rearrange("p (cb m) -> p cb m", cb=CB)
            wqk_src = wqk.rearrange("p (cb m) -> p cb m", cb=CB)

            nc.gpsimd.dma_start(out=wqk_dst[:, 0:2], in_=wqk_src[:, 0:2])
            nc.sync.dma_start(out=xt_dst[:, 0, 0:2], in_=xt_src[:, 0, 0:2])
            cos_sb = const.tile([128, T], MMD)
            nc.gpsimd.dma_start(out=cos_sb, in_=cosrep)
            sin_sb = const.tile([128, T], MMD)
            nc.gpsimd.dma_start(out=sin_sb, in_=sinsw)
            nc.sync.dma_start(out=xt_dst[:, 0, 2:5], in_=xt_src[:, 0, 2:5])
            nc.gpsimd.dma_start(out=wqk_dst[:, 2:5], in_=wqk_src[:, 2:5])
            nc.sync.dma_start(out=xt_dst[:, 0, 5:8], in_=xt_src[:, 0, 5:8])
            nc.gpsimd.dma_start(out=wqk_dst[:, 5:8], in_=wqk_src[:, 5:8])
            bqk_sb = const.tile([128, 4], F32)
            nc.gpsimd.dma_start(out=bqk_sb, in_=bqk)
            wv_sb = const.tile([128, CB * 256], MMD)
            nc.gpsimd.dma_start(out=wv_sb, in_=wv)
            for tci in range(1, NCH):
                nc.sync.dma_start(out=xt_dst[:, tci], in_=xt_src[:, tci])
            wproj_sb = const.tile([128, 2 * C], MMD)
            nc.gpsimd.dma_start(out=wproj_sb, in_=wproj)

            # ---- persistent intermediates ----------------------------------
            qt_sb = persist.tile([128, 2 * T], MMD)   # [Q01 | Q23], [d(2 heads), t]
            kt_sb = persist.tile([128, 2 * T], MMD)
            v_sb = persist.tile([128, NTB * 260], MMD)  # per j-block: 4x(64 v + 1 one)
            ctx0 = persist.tile([128, T], MMD)        # heads 0,1 ctxT
            ctx1 = persist.tile([128, T], MMD)        # heads 2,3 ctxT

            # ones columns of v_sb (denominator trick)
            nc.vector.memset(
                v_sb.rearrange("p (tb h d) -> p tb h d", tb=NTB, h=4)[:, :, :, 64:65],
                1.0,
            )

            # ---- work-unit emitters ----------------------------------------
            def emit_proj_mt(tci, mt):
                # q/k M-tiles: 0=Q(h0,h1) 1=Q(h2,h3) 2=K(h0,h1) 3=K(h2,h3)
                tsl = slice(tci * TCH, (tci + 1) * TCH)
                pq = ppj.tile([128, TCH], F32, tag="pj", name=f"pq{tci}_{mt}")
                for cb in range(CB):
                    nc.tensor.matmul(
                        pq,
                        lhsT=wqk_sb[:, cb * 512 + mt * 128 : cb * 512 + (mt + 1) * 128],
                        rhs=xt_slice(tci, cb),
                        start=(cb == 0),
                        stop=(cb == CB - 1),
                    )
                # rope: out = (pq+b)*cos + swap(pq+b)*sin  (sin sign-folded)
                m1 = work.tile([128, TCH], MMD, tag="m1", bufs=2, name=f"m1_{tci}_{mt}")
                nc.vector.scalar_tensor_tensor(
                    out=m1, in0=pq, scalar=bqk_sb[:, mt : mt + 1],
                    in1=cos_sb[:, tsl], op0=ALU.add, op1=ALU.mult,
                )
                swp = work.tile([128, TCH], MMD, tag="swp", bufs=2, name=f"swp{tci}_{mt}")
                if QB_ACT:
                    qb = work.tile([128, TCH], MMD, tag="qb", bufs=2, name=f"qb{tci}_{mt}")
                    nc.scalar.activation(
                        qb, pq, AF.Identity, bias=bqk_sb[:, mt : mt + 1], scale=1.0
                    )
                    for dst0, src0 in ((0, 32), (32, 0), (64, 96), (96, 64)):
                        nc.vector.tensor_mul(
                            swp[dst0 : dst0 + 32, :],
                            qb[src0 : src0 + 32, :],
                            sin_sb[src0 : src0 + 32, tsl],
                        )
                else:
                    # read pq straight from psum; bias rides the STT scalar
                    for dst0, src0 in ((0, 32), (32, 0), (64, 96), (96, 64)):
                        nc.vector.scalar_tensor_tensor(
                            out=swp[dst0 : dst0 + 32, :],
                            in0=pq[src0 : src0 + 32, :],
                            scalar=bqk_sb[src0 : src0 + 32, mt : mt + 1],
                            in1=sin_sb[src0 : src0 + 32, tsl],
                            op0=ALU.add, op1=ALU.mult,
                        )
                dest = qt_sb if mt < 2 else kt_sb
                dcol = (mt % 2) * T + tci * TCH
                nc.vector.tensor_add(dest[:, dcol : dcol + TCH], m1, swp)

            def emit_v_unit(tci, u):
                # v projection, natural layout [t, d]; 2 t-blocks per psum tile
                pv = ppj.tile([128, TCH], F32, tag="pj", name=f"pv{tci}_{u}")
                for q in range(2):
                    tb2 = u * 2 + q
                    for cb in range(CB):
                        nc.tensor.matmul(
                            pv[:, q * 256 : (q + 1) * 256],
                            lhsT=xt_slice(tci, cb, tb2 * 128, (tb2 + 1) * 128),
                            rhs=wv_sb[:, cb * 256 : (cb + 1) * 256],
                            start=(cb == 0),
                            stop=(cb == CB - 1),
                        )
                for q in range(2):
                    tb = tci * 4 + u * 2 + q
                    nc.vector.tensor_copy(
                        v_sb[:, tb * 260 : tb * 260 + 260].rearrange(
                            "p (h d) -> p h d", h=4
                        )[:, :, 0:64],
                        pv[:, q * 256 : (q + 1) * 256].rearrange(
                            "p (h d) -> p h d", h=4
                        ),
                    )

            def emit_outproj_unit(tci, tb2, tail=False):
                # output rows [tb*128, +128) x all 1024 cols (2 psum tiles)
                tb = tci * 4 + tb2
                ysb = work.tile([128, 1024], MMD, tag="ysb", bufs=2, name=f"ysb{tb}")
                for ncol in range(2):
                    yp = ppj.tile([128, TCH], F32, tag="pj", name=f"yp{tb}_{ncol}")
                    for cbp in range(2):
                        ctx_t = ctx0 if cbp == 0 else ctx1
                        nc.tensor.matmul(
                            yp,
                            lhsT=ctx_t[:, tb * 128 : (tb + 1) * 128],
                            rhs=wproj_sb[:, cbp * C + ncol * 512 : cbp * C + (ncol + 1) * 512],
                            start=(cbp == 0),
                            stop=(cbp == 1),
                        )
                    if tail:
                        # scalar engine is exp-free at the tail
                        nc.scalar.activation(
                            ysb[:, ncol * 512 : (ncol + 1) * 512], yp,
                            AF.Identity, scale=1.0,
                        )
                    else:
                        nc.vector.tensor_copy(ysb[:, ncol * 512 : (ncol + 1) * 512], yp)
                nc.sync.dma_start(out=yout[tb * 128 : (tb + 1) * 128, :], in_=ysb)

            def make_norm(pair, ici, ctxps, ctx_p):
                # ctx[d,i] = ctxps[d,i] / ctxps[64,i]: evict both heads'
                # denominator rows into one [2,512] tile, joint reciprocal
                # (same DVE wall as one row), broadcast, one multiply each.
                isl = slice(ici * TCH, (ici + 1) * TCH)

                def emit_norm():
                    for hh in range(2):
                        dn = work.tile([1, TCH], F32, tag="dnrow", bufs=2,
                                       name=f"dn{pair}_{ici}_{hh}")
                        nc.vector.tensor_copy(dn, ctxps[hh][64:65, :])
                        rc = work.tile([1, TCH], F32, tag="recip", bufs=2,
                                       name=f"rc{pair}_{ici}_{hh}")
                        nc.vector.reciprocal_approx_fast(out=rc, in_=dn)
                        bcast = work.tile([64, TCH], F32, tag="bcast", bufs=2,
                                          name=f"bc{pair}_{ici}_{hh}")
                        nc.gpsimd.partition_broadcast(bcast, rc)
                        nc.vector.tensor_mul(
                            ctx_p[hh * 64 : (hh + 1) * 64, isl],
                            ctxps[hh][0:64, :],
                            bcast,
                        )

                return emit_norm

            # ---- attention for one i-chunk, with filler injection ----------
            # st/at tiles are jb-major: [128 j, h0 512 i | h1 512 i].
            # Returns the deferred pair1-normalize closure for the caller to
            # inject into the next chunk's stream.
            def emit_attn(ici, fillers):
                njb = 4 * (ici + 1)
                jbs_left = 2 * njb  # njb j-blocks per pair, 2 pairs
                deferred = None
                for pair in range(2):
                    qt_p = qt_sb[:, pair * T : (pair + 1) * T]
                    kt_p = kt_sb[:, pair * T : (pair + 1) * T]
                    ctx_p = ctx0 if pair == 0 else ctx1
                    ctxps = [
                        pctx.tile([65, TCH], F32, tag="ctx", bufs=2, name=f"ctxp{pair}_{ici}_{hh}")
                        for hh in range(2)
                    ]

                    def emit_av(at, jb):
                        q = jb - 4 * ici  # >0 on trimmed diagonal j-blocks
                        c0 = q * 128 if (q > 0 and TRIM) else 0
                        for hh in range(2):
                            h_loc = pair * 2 + hh
                            nc.tensor.matmul(
                                ctxps[hh][:, c0:],
                                lhsT=v_sb[:, jb * 260 + h_loc * 65 : jb * 260 + (h_loc + 1) * 65],
                                rhs=at[:, hh * 512 + c0 : (hh + 1) * 512],
                                start=(jb == 0),
                                stop=(jb == njb - 1),
                            )

                    pending = []
                    for jb in range(njb):
                        q = jb - 4 * ici
                        c0 = q * 128 if (q > 0 and TRIM) else 0
                        st = pst.tile([128, 1024], F32, tag="st", name=f"st{pair}_{ici}_{jb}")
                        for hh in range(2):
                            nc.tensor.matmul(
                                st[:, hh * 512 + c0 : (hh + 1) * 512],
                                lhsT=kt_p[hh * 64 : (hh + 1) * 64, jb * 128 : (jb + 1) * 128],
                                rhs=qt_p[hh * 64 : (hh + 1) * 64, ici * TCH + c0 : (ici + 1) * TCH],
                                start=True,
                                stop=True,
                            )
                        # inject fillers to keep the PE fed while exp runs
                        if fillers:
                            n = -(-len(fillers) // jbs_left) if jbs_left else len(fillers)
                            for _ in range(min(n, len(fillers))):
                                fillers.pop(0)()
                        jbs_left -= 1
                        if len(pending) >= AV_DEPTH:
                            emit_av(*pending.pop(0))
                        at = attnp.tile([128, 1024], MMD, tag="attn", bufs=6, name=f"at{pair}_{ici}_{jb}")
                        if TRIM and q >= 1:
                            # left 128*q cols of each half are fully masked
                            # and AV-trimmed; one exp over both live ranges
                            at2 = at.rearrange("p (h i) -> p h i", h=2)
                            st2 = st.rearrange("p (h i) -> p h i", h=2)
                            nc.scalar.activation(
                                at2[:, :, c0:], st2[:, :, c0:], AF.Exp, scale=0.125
                            )
                        else:
                            nc.scalar.activation(at, st, AF.Exp, scale=0.125)
                        if q >= 0:
                            if TRIM:
                                # causal triangle: zero j>i in the 128x128
                                # diagonal block of both heads at once
                                # (iota = i - j = col - partition)
                                tri = at.rearrange("p (h i) -> p h i", h=2)[
                                    :, :, q * 128 : (q + 1) * 128
                                ]
                                nc.gpsimd.affine_select(
                                    out=tri, in_=tri,
                                    compare_op=ALU.is_ge, fill=0.0,
                                    base=0, channel_multiplier=-1,
                                    pattern=[[0, 2], [1, 128]],
                                )
                            else:
                                for hh in range(2):
                                    half_v = at[:, hh * 512 : (hh + 1) * 512]
                                    nc.gpsimd.affine_select(
                                        out=half_v, in_=half_v,
                                        compare_op=ALU.is_ge, fill=0.0,
                                        base=ici * TCH - jb * 128,
                                        channel_multiplier=-1,
                                        pattern=[[1, 512]],
                                    )
                        pending.append((at, jb))
                    for p in pending:
                        emit_av(*p)

                    # normalize is deferred: pair0's into pair1's stream,
                    # pair1's into the next chunk (returned to caller)
                    norm = make_norm(pair, ici, ctxps, ctx_p)
                    if pair == 0:
                        fillers.insert(0, norm)
                    else:
                        deferred = norm
                return deferred

            # ---- the pipelined schedule ------------------------------------
            # chunk 0 projection up front
            for emitter in [
                lambda: emit_proj_mt(0, 0), lambda: emit_proj_mt(0, 1),
                lambda: emit_v_unit(0, 0), lambda: emit_proj_mt(0, 2),
                lambda: emit_proj_mt(0, 3), lambda: emit_v_unit(0, 1),
            ]:
                emitter()

            def proj_units(t):
                return [
                    (lambda mt=mt: emit_proj_mt(t, mt)) for mt in range(4)
                ] + [
                    (lambda u=u: emit_v_unit(t, u)) for u in range(2)
                ]

            def outproj_units(t):
                return [(lambda tb2=tb2: emit_outproj_unit(t, tb2)) for tb2 in range(4)]

            # fillers: next chunk's projection during attn(k); output
            # projections pulled up as soon as their ctx chunk exists
            filler_plan = {
                0: proj_units(1),
                1: proj_units(2) + outproj_units(0),
                2: proj_units(3) + outproj_units(1),
                3: outproj_units(2),
            }
            deferred_norm = None
            for ici in range(NCH):
                fillers = filler_plan[ici]
                if deferred_norm is not None:
                    fillers.insert(0, deferred_norm)
                deferred_norm = emit_attn(ici, fillers)
                for f in fillers:  # leftovers
                    f()
            deferred_norm()
            for tb2 in range(4):
                emit_outproj_unit(NCH - 1, tb2, tail=True)

    nc.compile()
    return nc


def _prep_core_inputs(x, cos, sin, w_attn, b_attn, w_proj):
    """Build the 8 per-core input maps (host-side shard/reorder)."""
    import ml_dtypes
    mmnp = ml_dtypes.bfloat16
    x = np.asarray(x, dtype=np.float32)
    cos = np.asarray(cos, dtype=np.float32).reshape(T, HALF)
    sin = np.asarray(sin, dtype=np.float32).reshape(T, HALF)
    w_attn = np.asarray(w_attn, dtype=np.float32)
    b_attn = np.asarray(b_attn, dtype=np.float32)
    w_proj = np.asarray(w_proj, dtype=np.float32)

    cosT = np.ascontiguousarray(cos.T)               # [32, T]
    sinT = np.ascontiguousarray(sin.T)
    cosrep = np.tile(cosT, (4, 1)).astype(mmnp)      # [128, T]
    sin_sw = np.concatenate([sinT, -sinT, sinT, -sinT], axis=0).astype(mmnp)

    # xt rows: [p, tci, cb, t'] — identical to the device SBUF layout
    xts = []
    for b in range(B):
        xb = np.ascontiguousarray(x[b].T).astype(mmnp)      # [C, T]
        x4 = xb.reshape(CB, 128, NCH, TCH).transpose(1, 2, 0, 3)  # [p,tci,cb,t']
        xts.append(np.ascontiguousarray(x4).reshape(128, NCH * CB * TCH))

    def pack_rows(wmat, inner):
        # [CB*128, inner] -> [128, CB*inner] partition-major contiguous rows
        return np.ascontiguousarray(
            wmat.reshape(CB, 128, inner).transpose(1, 0, 2)
        ).reshape(128, CB * inner)

    in_maps = []
    for core in range(NCORES):
        b = core // 4
        g = core % 4
        heads = [4 * g + i for i in range(NH)]
        # q/k column blocks: M-tiles [Q(h0,h1), Q(h2,h3), K(h0,h1), K(h2,h3)]
        qcols, bq = [], []
        for mt, (base, hs) in enumerate(
            [(0, heads[0:2]), (0, heads[2:4]), (C, heads[0:2]), (C, heads[2:4])]
        ):
            cols = np.concatenate([np.arange(base + h * HS, base + (h + 1) * HS) for h in hs])
            qcols.append(cols)
            bq.append(b_attn[cols])
        wqk_c = pack_rows(w_attn[:, np.concatenate(qcols)].astype(mmnp), 512)
        bqk_c = np.stack(bq).T                                          # [128, 4]
        vcols = np.concatenate(
            [np.arange(2 * C + h * HS, 2 * C + (h + 1) * HS) for h in heads]
        )
        wv_c = pack_rows(w_attn[:, vcols].astype(mmnp), 256)
        wproj_c = np.ascontiguousarray(
            w_proj[g * 256 : (g + 1) * 256, :].astype(mmnp)
            .reshape(2, 128, C).transpose(1, 0, 2)
        ).reshape(128, 2 * C)
        in_maps.append(
            {
                "xt": xts[b],
                "wqk": wqk_c,
                "wv": wv_c,
                "wproj": wproj_c,
                "bqk": np.ascontiguousarray(bqk_c),
                "cosrep": np.ascontiguousarray(cosrep),
                "sinsw": np.ascontiguousarray(sin_sw),
            }
        )
    return in_maps


def kernel(x, cos, sin, w_attn, b_attn, w_proj, b_proj, _want_trace=False):
    if "nc" not in _CACHED:
        _CACHED["nc"] = _build_nc()
    nc = _CACHED["nc"]
    in_maps = _prep_core_inputs(x, cos, sin, w_attn, b_attn, w_proj)
    res = run_bass_kernel_spmd(
        nc, in_maps, core_ids=list(range(NCORES)), trace=_want_trace
    )
    _CACHED["last_result"] = res
    b_proj = np.asarray(b_proj, dtype=np.float32)
    # v-bias folds out of attention (softmax rows sum to 1): it contributes a
    # constant b_v @ w_proj to every output row, added here with b_proj.
    bv = np.asarray(b_attn, dtype=np.float32)[2 * C : 3 * C]
    bias_full = b_proj + bv @ np.asarray(w_proj, dtype=np.float32)
    out = np.empty((B, T, C), np.float32)
    for b in range(B):
        acc = res.results[b * 4]["yout"].astype(np.float32)
        for g in range(1, 4):
            acc += res.results[b * 4 + g]["yout"].astype(np.float32)
        out[b] = acc + bias_full[None, :]
    return out


# revision 10
# speedup vs baseline: 1.0371x; 1.0371x over previous
"""Causal self-attention (B=2, T=2048, C=1024, H=16) on 8 TRN2 NeuronCores.

Sharding: 8 cores = 2 batches x 4 head-groups (4 heads each).
Each core computes qkv projection for its heads, attention, and a partial
output projection (its rows of w_proj); the host sums the 4 partials per
batch and adds b_proj.

v3 (from v2's globally software-pipelined single stream):
  - xt SBUF layout is chunk-contiguous [tci][cb][t'] matching the host
    pack, so each chunk DMA is one 8KB span per partition (the v2 layout
    forced 1KB-granular descriptors; ~12us of descriptor-gen per chunk).
  - Input DMAs split across sync (xt) + gpsimd (weights) queues,
    first-needed-first, finer granularity so matmuls chase arrival.
  - Scalar engine runs (almost) only exp: the qb Identity-activation is
    gone - rope swap-muls read pq straight from PSUM via STT with the
    bias folded into the scalar operand.
  - Diagonal tiles: one exp instr (2-level AP over both heads' live
    ranges) and one affine_select per tile instead of two.
  - Softmax normalize: partition_broadcast of the denominator row
    (straight from PSUM), reciprocal on the broadcast [64,512] tile, one
    multiply; emission deferred into the NEXT pair's stream so it can't
    head-of-line-block the DVE queue (v2 lost ~6us to that at the
    attn(0)->attn(1) boundary).
  - Filler rebalance: outproj(0)/(1) pulled up into attn(1)/(2); tail
    outproj PSUM evictions on the scalar engine (idle at the tail).
"""

import sys
import os

for _p in ("/opt/trn_rl_repo", "/root/.axon_site/_ro/trn_rl_repo"):
    if os.path.isdir(_p) and _p not in sys.path:
        sys.path.insert(0, _p)

import numpy as np
import concourse.bass as bass
import concourse.mybir as mybir
import concourse.tile as tile
from concourse import bacc
from concourse.bass_utils import run_bass_kernel_spmd

B, T, C, H = 2, 2048, 1024, 16
HS = C // H          # 64
HALF = HS // 2       # 32
NCORES = 8
NH = 4               # heads per core
TCH = 512            # t-chunk for projections / i-chunk for attention
NCH = T // TCH       # 4 chunks
CB = C // 128        # 8 contraction blocks
NTB = T // 128       # 16 t/j blocks
F32 = mybir.dt.float32
BF16 = mybir.dt.bfloat16
MMD = BF16
AF = mybir.ActivationFunctionType
ALU = mybir.AluOpType

TRIM = os.environ.get("KTRIM", "1") == "1"       # causal QK/exp/AV trims
AV_DEPTH = int(os.environ.get("KAVDEPTH", "2"))  # AV pipeline lookahead
# qb eviction on scalar: keeping it OFF makes the rope swap-muls read pq
# from PSUM via STT, which measured ~2x slower per op AND slowed exp by
# ~20% through PSUM port contention. Keep ON.
QB_ACT = os.environ.get("KQBACT", "1") == "1"
KDIV = os.environ.get("KDIV", "0") == "1"    # DVE divide: NOT a valid TT ISA op, keep 0

_CACHED = {}


def _build_nc():
    nc = bacc.Bacc("TRN2", target_bir_lowering=False, debug=False)

    # dram layouts are partition-major with long contiguous rows so each
    # DMA descriptor moves a big span; host prepacks accordingly
    xt = nc.dram_tensor("xt", [128, NCH * CB * TCH], MMD, kind="ExternalInput").ap()
    wqk = nc.dram_tensor("wqk", [128, CB * 512], MMD, kind="ExternalInput").ap()
    wv = nc.dram_tensor("wv", [128, CB * 256], MMD, kind="ExternalInput").ap()
    wproj = nc.dram_tensor("wproj", [128, 2 * C], MMD, kind="ExternalInput").ap()
    bqk = nc.dram_tensor("bqk", [128, 4], F32, kind="ExternalInput").ap()
    cosrep = nc.dram_tensor("cosrep", [64, T], MMD, kind="ExternalInput").ap()
    sinsw = nc.dram_tensor("sinsw", [64, T], MMD, kind="ExternalInput").ap()
    yout = nc.dram_tensor("yout", [T, C], MMD, kind="ExternalOutput").ap()

    with tile.TileContext(nc) as tc:
        with (
            tc.tile_pool(name="const", bufs=1) as const,
            tc.tile_pool(name="persist", bufs=1) as persist,
            tc.tile_pool(name="work", bufs=2) as work,
            tc.tile_pool(name="attnp", bufs=6) as attnp,
            tc.tile_pool(name="pst", bufs=2, space="PSUM") as pst,
            tc.tile_pool(name="ppj", bufs=2, space="PSUM") as ppj,
            tc.tile_pool(name="pctx", bufs=1, space="PSUM") as pctx,
        ):
            # ---- DMA: xt on sync, weights on gpsimd, first-needed first.
            # xt_sb columns are [tci][cb][t'] = exactly the dram row layout,
            # so every transfer is long-contiguous on both sides.
            wqk_sb = const.tile([128, CB * 512], MMD)
            xt_sb = const.tile([128, NCH * CB * TCH], MMD)  # all chunks resident

            def xt_slice(tci, cb, t0=0, t1=TCH):
                base = tci * CB * TCH + cb * TCH
                return xt_sb[:, base + t0 : base + t1]

            xt_dst = xt_sb.rearrange("p (tci cb t) -> p tci cb t", tci=NCH, cb=CB)
            xt_src = xt.rearrange("p (tci cb t) -> p tci cb t", tci=NCH, cb=CB)
            wqk_dst = wqk_sb.rearrange("p (cb m) -> p cb m", cb=CB)
            wqk_src = wqk.rearrange("p (cb m) -> p cb m", cb=CB)

            nc.gpsimd.dma_start(out=wqk_dst[:, 0:2], in_=wqk_src[:, 0:2])
            nc.sync.dma_start(out=xt_dst[:, 0, 0:2], in_=xt_src[:, 0, 0:2])
            nc.gpsimd.dma_start(out=wqk_dst[:, 2:5], in_=wqk_src[:, 2:5])
            nc.sync.dma_start(out=xt_dst[:, 0, 2:5], in_=xt_src[:, 0, 2:5])
            nc.gpsimd.dma_start(out=wqk_dst[:, 5:8], in_=wqk_src[:, 5:8])
            nc.sync.dma_start(out=xt_dst[:, 0, 5:8], in_=xt_src[:, 0, 5:8])
            # cos/sin ship as [64, T] (the sign-folded sin repeats with
            # period 64) and are replicated on-chip: saves 1MB of startup
            # wire time
            cos_sb = const.tile([128, T], MMD)
            sin_sb = const.tile([128, T], MMD)
            nc.gpsimd.dma_start(out=cos_sb[0:64, :], in_=cosrep)
            nc.gpsimd.dma_start(out=sin_sb[0:64, :], in_=sinsw)
            nc.scalar.dma_start(out=cos_sb[64:128, :], in_=cos_sb[0:64, :])
            nc.scalar.dma_start(out=sin_sb[64:128, :], in_=sin_sb[0:64, :])
            bqk_sb = const.tile([128, 4], F32)
            nc.gpsimd.dma_start(out=bqk_sb, in_=bqk)
            wv_sb = const.tile([128, CB * 256], MMD)
            nc.gpsimd.dma_start(out=wv_sb, in_=wv)
            for tci in range(1, NCH):
                nc.sync.dma_start(out=xt_dst[:, tci], in_=xt_src[:, tci])
            wproj_sb = const.tile([128, 2 * C], MMD)
            nc.gpsimd.dma_start(out=wproj_sb, in_=wproj)

            # preload the gpsimd ucode library that partition_broadcast
            # lives in while gpsimd is otherwise idle (the lazy load at
            # first use measured ~5us of dead time mid-kernel)
            pb_src = work.tile([1, 8], F32, tag="pbs", bufs=1, name="pb_src")
            pb_dst = work.tile([64, 8], F32, tag="pbd", bufs=1, name="pb_dst")
            nc.vector.memset(pb_src, 1.0)
            nc.gpsimd.partition_broadcast(pb_dst, pb_src)

            # ---- persistent intermediates ----------------------------------
            qt_sb = persist.tile([128, 2 * T], MMD)   # [Q01 | Q23], [d(2 heads), t]
            kt_sb = persist.tile([128, 2 * T], MMD)
            v_sb = persist.tile([128, NTB * 260], MMD)  # per j-block: 4x(64 v + 1 one)
            ctx0 = persist.tile([128, T], MMD)        # heads 0,1 ctxT
            ctx1 = persist.tile([128, T], MMD)        # heads 2,3 ctxT

            # ones columns of v_sb (denominator trick)
            nc.vector.memset(
                v_sb.rearrange("p (tb h d) -> p tb h d", tb=NTB, h=4)[:, :, :, 64:65],
                1.0,
            )

            # ---- work-unit emitters ----------------------------------------
            def emit_proj_mt(tci, mt):
                # q/k M-tiles: 0=Q(h0,h1) 1=Q(h2,h3) 2=K(h0,h1) 3=K(h2,h3)
                tsl = slice(tci * TCH, (tci + 1) * TCH)
                pq = ppj.tile([128, TCH], F32, tag="pj", name=f"pq{tci}_{mt}")
                for cb in range(CB):
                    nc.tensor.matmul(
                        pq,
                        lhsT=wqk_sb[:, cb * 512 + mt * 128 : cb * 512 + (mt + 1) * 128],
                        rhs=xt_slice(tci, cb),
                        start=(cb == 0),
                        stop=(cb == CB - 1),
                    )
                # rope: out = (pq+b)*cos + swap(pq+b)*sin  (sin sign-folded)
                m1 = work.tile([128, TCH], MMD, tag="m1", bufs=2, name=f"m1_{tci}_{mt}")
                nc.vector.scalar_tensor_tensor(
                    out=m1, in0=pq, scalar=bqk_sb[:, mt : mt + 1],
                    in1=cos_sb[:, tsl], op0=ALU.add, op1=ALU.mult,
                )
                swp = work.tile([128, TCH], MMD, tag="swp", bufs=2, name=f"swp{tci}_{mt}")
                if QB_ACT:
                    qb = work.tile([128, TCH], MMD, tag="qb", bufs=2, name=f"qb{tci}_{mt}")
                    nc.scalar.activation(
                        qb, pq, AF.Identity, bias=bqk_sb[:, mt : mt + 1], scale=1.0
                    )
                    for dst0, src0 in ((0, 32), (32, 0), (64, 96), (96, 64)):
                        nc.vector.tensor_mul(
                            swp[dst0 : dst0 + 32, :],
                            qb[src0 : src0 + 32, :],
                            sin_sb[src0 : src0 + 32, tsl],
                        )
                else:
                    # read pq straight from psum; bias rides the STT scalar
                    for dst0, src0 in ((0, 32), (32, 0), (64, 96), (96, 64)):
                        nc.vector.scalar_tensor_tensor(
                            out=swp[dst0 : dst0 + 32, :],
                            in0=pq[src0 : src0 + 32, :],
                            scalar=bqk_sb[src0 : src0 + 32, mt : mt + 1],
                            in1=sin_sb[src0 : src0 + 32, tsl],
                            op0=ALU.add, op1=ALU.mult,
                        )
                dest = qt_sb if mt < 2 else kt_sb
                dcol = (mt % 2) * T + tci * TCH
                nc.vector.tensor_add(dest[:, dcol : dcol + TCH], m1, swp)

            def emit_v_unit(tci, u):
                # v projection, natural layout [t, d]; 2 t-blocks per psum tile
                pv = ppj.tile([128, TCH], F32, tag="pj", name=f"pv{tci}_{u}")
                for q in range(2):
                    tb2 = u * 2 + q
                    for cb in range(CB):
                        nc.tensor.matmul(
                            pv[:, q * 256 : (q + 1) * 256],
                            lhsT=xt_slice(tci, cb, tb2 * 128, (tb2 + 1) * 128),
                            rhs=wv_sb[:, cb * 256 : (cb + 1) * 256],
                            start=(cb == 0),
                            stop=(cb == CB - 1),
                        )
                for q in range(2):
                    tb = tci * 4 + u * 2 + q
                    nc.vector.tensor_copy(
                        v_sb[:, tb * 260 : tb * 260 + 260].rearrange(
                            "p (h d) -> p h d", h=4
                        )[:, :, 0:64],
                        pv[:, q * 256 : (q + 1) * 256].rearrange(
                            "p (h d) -> p h d", h=4
                        ),
                    )

            def emit_outproj_unit(tci, tb2, tail=False):
                # output rows [tb*128, +128) x all 1024 cols (2 psum tiles)
                tb = tci * 4 + tb2
                ysb = work.tile([128, 1024], MMD, tag="ysb", bufs=2, name=f"ysb{tb}")
                for ncol in range(2):
                    yp = ppj.tile([128, TCH], F32, tag="pj", name=f"yp{tb}_{ncol}")
                    for cbp in range(2):
                        ctx_t = ctx0 if cbp == 0 else ctx1
                        nc.tensor.matmul(
                            yp,
                            lhsT=ctx_t[:, tb * 128 : (tb + 1) * 128],
                            rhs=wproj_sb[:, cbp * C + ncol * 512 : cbp * C + (ncol + 1) * 512],
                            start=(cbp == 0),
                            stop=(cbp == 1),
                        )
                    if tail:
                        # scalar engine is exp-free at the tail
                        nc.scalar.activation(
                            ysb[:, ncol * 512 : (ncol + 1) * 512], yp,
                            AF.Identity, scale=1.0,
                        )
                    else:
                        nc.vector.tensor_copy(ysb[:, ncol * 512 : (ncol + 1) * 512], yp)
                nc.sync.dma_start(out=yout[tb * 128 : (tb + 1) * 128, :], in_=ysb)

            def make_norm(pair, ici, ctxps, ctx_p):
                # ctx[d,i] = ctxps[d,i] / ctxps[64,i]: evict both heads'
                # denominator rows into one [2,512] tile, joint reciprocal
                # (same DVE wall as one row), broadcast, one multiply each.
                isl = slice(ici * TCH, (ici + 1) * TCH)

                def emit_norm():
                    for hh in range(2):
                        dn = work.tile([1, TCH], F32, tag="dnrow", bufs=2,
                                       name=f"dn{pair}_{ici}_{hh}")
                        nc.vector.tensor_copy(dn, ctxps[hh][64:65, :])
                        bcast = work.tile([64, TCH], F32, tag="bcast", bufs=2,
                                          name=f"bc{pair}_{ici}_{hh}")
                        nc.gpsimd.partition_broadcast(bcast, dn)
                        if KDIV:
                            nc.vector.tensor_tensor(
                                out=ctx_p[hh * 64 : (hh + 1) * 64, isl],
                                in0=ctxps[hh][0:64, :],
                                in1=bcast,
                                op=ALU.divide,
                            )
                        else:
                            rc = work.tile([64, TCH], F32, tag="recip", bufs=2,
                                           name=f"rc{pair}_{ici}_{hh}")
                            nc.vector.reciprocal_approx_fast(out=rc, in_=bcast)
                            nc.vector.tensor_mul(
                                ctx_p[hh * 64 : (hh + 1) * 64, isl],
                                ctxps[hh][0:64, :],
                                rc,
                            )

                return emit_norm

            # ---- attention for one i-chunk, with filler injection ----------
            # st/at tiles are jb-major: [128 j, h0 512 i | h1 512 i].
            # Returns the deferred pair1-normalize closure for the caller to
            # inject into the next chunk's stream.
            def emit_attn(ici, fillers):
                njb = 4 * (ici + 1)
                jbs_left = 2 * njb  # njb j-blocks per pair, 2 pairs
                deferred = None
                for pair in range(2):
                    qt_p = qt_sb[:, pair * T : (pair + 1) * T]
                    kt_p = kt_sb[:, pair * T : (pair + 1) * T]
                    ctx_p = ctx0 if pair == 0 else ctx1
                    ctxps = [
                        pctx.tile([65, TCH], F32, tag="ctx", bufs=2, name=f"ctxp{pair}_{ici}_{hh}")
                        for hh in range(2)
                    ]

                    def emit_av(at, jb):
                        q = jb - 4 * ici  # >0 on trimmed diagonal j-blocks
                        c0 = q * 128 if (q > 0 and TRIM) else 0
                        for hh in range(2):
                            h_loc = pair * 2 + hh
                            nc.tensor.matmul(
                                ctxps[hh][:, c0:],
                                lhsT=v_sb[:, jb * 260 + h_loc * 65 : jb * 260 + (h_loc + 1) * 65],
                                rhs=at[:, hh * 512 + c0 : (hh + 1) * 512],
                                start=(jb == 0),
                                stop=(jb == njb - 1),
                            )

                    pending = []
                    for jb in range(njb):
                        q = jb - 4 * ici
                        c0 = q * 128 if (q > 0 and TRIM) else 0
                        st = pst.tile([128, 1024], F32, tag="st", name=f"st{pair}_{ici}_{jb}")
                        for hh in range(2):
                            nc.tensor.matmul(
                                st[:, hh * 512 + c0 : (hh + 1) * 512],
                                lhsT=kt_p[hh * 64 : (hh + 1) * 64, jb * 128 : (jb + 1) * 128],
                                rhs=qt_p[hh * 64 : (hh + 1) * 64, ici * TCH + c0 : (ici + 1) * TCH],
                                start=True,
                                stop=True,
                            )
                        # inject fillers to keep the PE fed while exp runs
                        if fillers:
                            n = -(-len(fillers) // jbs_left) if jbs_left else len(fillers)
                            for _ in range(min(n, len(fillers))):
                                fillers.pop(0)()
                        jbs_left -= 1
                        if len(pending) >= AV_DEPTH:
                            emit_av(*pending.pop(0))
                        at = attnp.tile([128, 1024], MMD, tag="attn", bufs=6, name=f"at{pair}_{ici}_{jb}")
                        if TRIM and q >= 1:
                            # left 128*q cols of each half are fully masked
                            # and AV-trimmed; one exp over both live ranges
                            at2 = at.rearrange("p (h i) -> p h i", h=2)
                            st2 = st.rearrange("p (h i) -> p h i", h=2)
                            nc.scalar.activation(
                                at2[:, :, c0:], st2[:, :, c0:], AF.Exp, scale=0.125
                            )
                        else:
                            nc.scalar.activation(at, st, AF.Exp, scale=0.125)
                        if q >= 0:
                            if TRIM:
                                # causal triangle: zero j>i in the 128x128
                                # diagonal block of both heads at once
                                # (iota = i - j = col - partition)
                                tri = at.rearrange("p (h i) -> p h i", h=2)[
                                    :, :, q * 128 : (q + 1) * 128
                                ]
                                nc.gpsimd.affine_select(
                                    out=tri, in_=tri,
                                    compare_op=ALU.is_ge, fill=0.0,
                                    base=0, channel_multiplier=-1,
                                    pattern=[[0, 2], [1, 128]],
                                )
                            else:
                                for hh in range(2):
                                    half_v = at[:, hh * 512 : (hh + 1) * 512]
                                    nc.gpsimd.affine_select(
                                        out=half_v, in_=half_v,
                                        compare_op=ALU.is_ge, fill=0.0,
                                        base=ici * TCH - jb * 128,
                                        channel_multiplier=-1,
                                        pattern=[[1, 512]],
                                    )
                        pending.append((at, jb))
                    for p in pending:
                        emit_av(*p)

                    # normalize is deferred: pair0's into pair1's stream,
                    # pair1's into the next chunk (returned to caller)
                    norm = make_norm(pair, ici, ctxps, ctx_p)
                    if pair == 0:
                        fillers.insert(0, norm)
                    else:
                        deferred = norm
                return deferred

            # ---- the pipelined schedule ------------------------------------
            # chunk 0 projection up front
            for emitter in [
                lambda: emit_proj_mt(0, 0), lambda: emit_proj_mt(0, 2),
                lambda: emit_v_unit(0, 0), lambda: emit_proj_mt(0, 1),
                lambda: emit_proj_mt(0, 3), lambda: emit_v_unit(0, 1),
            ]:
                emitter()

            def proj_units(t):
                return [
                    (lambda mt=mt: emit_proj_mt(t, mt)) for mt in range(4)
                ] + [
                    (lambda u=u: emit_v_unit(t, u)) for u in range(2)
                ]

            def outproj_units(t):
                return [(lambda tb2=tb2: emit_outproj_unit(t, tb2)) for tb2 in range(4)]

            # fillers: next chunk's projection during attn(k); output
            # projections pulled up as soon as their ctx chunk exists
            filler_plan = {
                0: proj_units(1),
                1: proj_units(2) + outproj_units(0),
                2: proj_units(3) + outproj_units(1),
                3: outproj_units(2),
            }
            deferred_norm = None
            for ici in range(NCH):
                fillers = filler_plan[ici]
                if deferred_norm is not None:
                    fillers.insert(0, deferred_norm)
                deferred_norm = emit_attn(ici, fillers)
                for f in fillers:  # leftovers
                    f()
            deferred_norm()
            for tb2 in range(4):
                emit_outproj_unit(NCH - 1, tb2, tail=True)

    nc.compile()
    return nc


def _prep_core_inputs(x, cos, sin, w_attn, b_attn, w_proj):
    """Build the 8 per-core input maps (host-side shard/reorder)."""
    import ml_dtypes
    mmnp = ml_dtypes.bfloat16
    x = np.asarray(x, dtype=np.float32)
    cos = np.asarray(cos, dtype=np.float32).reshape(T, HALF)
    sin = np.asarray(sin, dtype=np.float32).reshape(T, HALF)
    w_attn = np.asarray(w_attn, dtype=np.float32)
    b_attn = np.asarray(b_attn, dtype=np.float32)
    w_proj = np.asarray(w_proj, dtype=np.float32)

    cosT = np.ascontiguousarray(cos.T)               # [32, T]
    sinT = np.ascontiguousarray(sin.T)
    cosrep = np.tile(cosT, (2, 1)).astype(mmnp)      # [64, T]
    sin_sw = np.concatenate([sinT, -sinT], axis=0).astype(mmnp)  # [64, T]

    # xt rows: [p, tci, cb, t'] — identical to the device SBUF layout
    xts = []
    for b in range(B):
        xb = np.ascontiguousarray(x[b].T).astype(mmnp)      # [C, T]
        x4 = xb.reshape(CB, 128, NCH, TCH).transpose(1, 2, 0, 3)  # [p,tci,cb,t']
        xts.append(np.ascontiguousarray(x4).reshape(128, NCH * CB * TCH))

    def pack_rows(wmat, inner):
        # [CB*128, inner] -> [128, CB*inner] partition-major contiguous rows
        return np.ascontiguousarray(
            wmat.reshape(CB, 128, inner).transpose(1, 0, 2)
        ).reshape(128, CB * inner)

    in_maps = []
    for core in range(NCORES):
        b = core // 4
        g = core % 4
        heads = [4 * g + i for i in range(NH)]
        # q/k column blocks: M-tiles [Q(h0,h1), Q(h2,h3), K(h0,h1), K(h2,h3)]
        qcols, bq = [], []
        for mt, (base, hs) in enumerate(
            [(0, heads[0:2]), (0, heads[2:4]), (C, heads[0:2]), (C, heads[2:4])]
        ):
            cols = np.concatenate([np.arange(base + h * HS, base + (h + 1) * HS) for h in hs])
            qcols.append(cols)
            bq.append(b_attn[cols])
        wqk_c = pack_rows(w_attn[:, np.concatenate(qcols)].astype(mmnp), 512)
        bqk_c = np.stack(bq).T                                          # [128, 4]
        vcols = np.concatenate(
            [np.arange(2 * C + h * HS, 2 * C + (h + 1) * HS) for h in heads]
        )
        wv_c = pack_rows(w_attn[:, vcols].astype(mmnp), 256)
        wproj_c = np.ascontiguousarray(
            w_proj[g * 256 : (g + 1) * 256, :].astype(mmnp)
            .reshape(2, 128, C).transpose(1, 0, 2)
        ).reshape(128, 2 * C)
        in_maps.append(
            {
                "xt": xts[b],
                "wqk": wqk_c,
                "wv": wv_c,
                "wproj": wproj_c,
                "bqk": np.ascontiguousarray(bqk_c),
                "cosrep": np.ascontiguousarray(cosrep),
                "sinsw": np.ascontiguousarray(sin_sw),
            }
        )
    return in_maps


def kernel(x, cos, sin, w_attn, b_attn, w_proj, b_proj, _want_trace=False):
    if "nc" not in _CACHED:
        _CACHED["nc"] = _build_nc()
    nc = _CACHED["nc"]
    in_maps = _prep_core_inputs(x, cos, sin, w_attn, b_attn, w_proj)
    res = run_bass_kernel_spmd(
        nc, in_maps, core_ids=list(range(NCORES)), trace=_want_trace
    )
    _CACHED["last_result"] = res
    b_proj = np.asarray(b_proj, dtype=np.float32)
    # v-bias folds out of attention (softmax rows sum to 1): it contributes a
    # constant b_v @ w_proj to every output row, added here with b_proj.
    bv = np.asarray(b_attn, dtype=np.float32)[2 * C : 3 * C]
    bias_full = b_proj + bv @ np.asarray(w_proj, dtype=np.float32)
    out = np.empty((B, T, C), np.float32)
    for b in range(B):
        acc = res.results[b * 4]["yout"].astype(np.float32)
        for g in range(1, 4):
            acc += res.results[b * 4 + g]["yout"].astype(np.float32)
        out[b] = acc + bias_full[None, :]
    return out


# revision 12
# speedup vs baseline: 1.0468x; 1.0093x over previous
"""Causal self-attention (B=2, T=2048, C=1024, H=16) on 8 TRN2 NeuronCores.

Sharding: 8 cores = 2 batches x 4 head-groups (4 heads each).
Each core computes qkv projection for its heads, attention, and a partial
output projection (its rows of w_proj); the host sums the 4 partials per
batch and adds b_proj.

v3 (from v2's globally software-pipelined single stream):
  - xt SBUF layout is chunk-contiguous [tci][cb][t'] matching the host
    pack, so each chunk DMA is one 8KB span per partition (the v2 layout
    forced 1KB-granular descriptors; ~12us of descriptor-gen per chunk).
  - Input DMAs split across sync (xt) + gpsimd (weights) queues,
    first-needed-first, finer granularity so matmuls chase arrival.
  - Scalar engine runs (almost) only exp: the qb Identity-activation is
    gone - rope swap-muls read pq straight from PSUM via STT with the
    bias folded into the scalar operand.
  - Diagonal tiles: one exp instr (2-level AP over both heads' live
    ranges) and one affine_select per tile instead of two.
  - Softmax normalize: partition_broadcast of the denominator row
    (straight from PSUM), reciprocal on the broadcast [64,512] tile, one
    multiply; emission deferred into the NEXT pair's stream so it can't
    head-of-line-block the DVE queue (v2 lost ~6us to that at the
    attn(0)->attn(1) boundary).
  - Filler rebalance: outproj(0)/(1) pulled up into attn(1)/(2); tail
    outproj PSUM evictions on the scalar engine (idle at the tail).
"""

import sys
import os

for _p in ("/opt/trn_rl_repo", "/root/.axon_site/_ro/trn_rl_repo"):
    if os.path.isdir(_p) and _p not in sys.path:
        sys.path.insert(0, _p)

import numpy as np
import concourse.bass as bass
import concourse.mybir as mybir
import concourse.tile as tile
from concourse import bacc
from concourse.bass_utils import run_bass_kernel_spmd

B, T, C, H = 2, 2048, 1024, 16
HS = C // H          # 64
HALF = HS // 2       # 32
NCORES = 8
NH = 4               # heads per core
TCH = 512            # t-chunk for projections / i-chunk for attention
NCH = T // TCH       # 4 chunks
CB = C // 128        # 8 contraction blocks
NTB = T // 128       # 16 t/j blocks
F32 = mybir.dt.float32
BF16 = mybir.dt.bfloat16
MMD = BF16
AF = mybir.ActivationFunctionType
ALU = mybir.AluOpType

TRIM = os.environ.get("KTRIM", "1") == "1"       # causal QK/exp/AV trims
AV_DEPTH = int(os.environ.get("KAVDEPTH", "2"))  # AV pipeline lookahead
# qb eviction on scalar: keeping it OFF makes the rope swap-muls read pq
# from PSUM via STT, which measured ~2x slower per op AND slowed exp by
# ~20% through PSUM port contention. Keep ON.
QB_ACT = os.environ.get("KQBACT", "1") == "1"
KDIV = os.environ.get("KDIV", "0") == "1"    # DVE divide: NOT a valid TT ISA op, keep 0

_CACHED = {}


def _build_nc():
    nc = bacc.Bacc("TRN2", target_bir_lowering=False, debug=False)

    # dram layouts are partition-major with long contiguous rows so each
    # DMA descriptor moves a big span; host prepacks accordingly
    xt = nc.dram_tensor("xt", [128, NCH * CB * TCH], MMD, kind="ExternalInput").ap()
    wqk = nc.dram_tensor("wqk", [128, CB * 512], MMD, kind="ExternalInput").ap()
    wv = nc.dram_tensor("wv", [128, CB * 256], MMD, kind="ExternalInput").ap()
    wproj = nc.dram_tensor("wproj", [128, 2 * C], MMD, kind="ExternalInput").ap()
    bqk = nc.dram_tensor("bqk", [128, 4], F32, kind="ExternalInput").ap()
    cosrep = nc.dram_tensor("cosrep", [64, T], MMD, kind="ExternalInput").ap()
    sinsw = nc.dram_tensor("sinsw", [64, T], MMD, kind="ExternalInput").ap()
    yout = nc.dram_tensor("yout", [T, C], MMD, kind="ExternalOutput").ap()

    with tile.TileContext(nc) as tc:
        with (
            tc.tile_pool(name="const", bufs=1) as const,
            tc.tile_pool(name="persist", bufs=1) as persist,
            tc.tile_pool(name="work", bufs=2) as work,
            tc.tile_pool(name="attnp", bufs=6) as attnp,
            tc.tile_pool(name="pst", bufs=2, space="PSUM") as pst,
            tc.tile_pool(name="ppj", bufs=2, space="PSUM") as ppj,
            tc.tile_pool(name="pctx", bufs=1, space="PSUM") as pctx,
        ):
            # ---- DMA: xt on sync, weights on gpsimd, first-needed first.
            # xt_sb columns are [tci][cb][t'] = exactly the dram row layout,
            # so every transfer is long-contiguous on both sides.
            wqk_sb = const.tile([128, CB * 512], MMD)
            xt_sb = const.tile([128, NCH * CB * TCH], MMD)  # all chunks resident

            def xt_slice(tci, cb, t0=0, t1=TCH):
                base = tci * CB * TCH + cb * TCH
                return xt_sb[:, base + t0 : base + t1]

            xt_dst = xt_sb.rearrange("p (tci cb t) -> p tci cb t", tci=NCH, cb=CB)
            xt_src = xt.rearrange("p (tci cb t) -> p tci cb t", tci=NCH, cb=CB)
            wqk_dst = wqk_sb.rearrange("p (cb m) -> p cb m", cb=CB)
            wqk_src = wqk.rearrange("p (cb m) -> p cb m", cb=CB)

            nc.gpsimd.dma_start(out=wqk_dst[:, 0:2], in_=wqk_src[:, 0:2])
            nc.sync.dma_start(out=xt_dst[:, 0, 0:2], in_=xt_src[:, 0, 0:2])
            nc.gpsimd.dma_start(out=wqk_dst[:, 2:5], in_=wqk_src[:, 2:5])
            nc.sync.dma_start(out=xt_dst[:, 0, 2:5], in_=xt_src[:, 0, 2:5])
            nc.gpsimd.dma_start(out=wqk_dst[:, 5:8], in_=wqk_src[:, 5:8])
            nc.sync.dma_start(out=xt_dst[:, 0, 5:8], in_=xt_src[:, 0, 5:8])
            # cos/sin ship as [64, T] (the sign-folded sin repeats with
            # period 64) and are replicated on-chip: saves 1MB of startup
            # wire time
            cos_sb = const.tile([128, T], MMD)
            sin_sb = const.tile([128, T], MMD)
            nc.gpsimd.dma_start(out=cos_sb[0:64, :], in_=cosrep)
            nc.gpsimd.dma_start(out=sin_sb[0:64, :], in_=sinsw)
            nc.scalar.dma_start(out=cos_sb[64:128, :], in_=cos_sb[0:64, :])
            nc.scalar.dma_start(out=sin_sb[64:128, :], in_=sin_sb[0:64, :])
            wv_sb = const.tile([128, CB * 256], MMD)
            nc.gpsimd.dma_start(out=wv_sb, in_=wv)
            bqk_sb = const.tile([128, 4], F32)
            nc.gpsimd.dma_start(out=bqk_sb, in_=bqk)
            for tci in range(1, NCH):
                nc.sync.dma_start(out=xt_dst[:, tci], in_=xt_src[:, tci])
            wproj_sb = const.tile([128, 2 * C], MMD)
            nc.gpsimd.dma_start(out=wproj_sb, in_=wproj)

            # preload the gpsimd ucode library that partition_broadcast
            # lives in while gpsimd is otherwise idle (the lazy load at
            # first use measured ~5us of dead time mid-kernel)
            pb_src = work.tile([1, 8], F32, tag="pbs", bufs=1, name="pb_src")
            pb_dst = work.tile([64, 8], F32, tag="pbd", bufs=1, name="pb_dst")
            nc.vector.memset(pb_src, 1.0)
            nc.gpsimd.partition_broadcast(pb_dst, pb_src)

            # ---- persistent intermediates ----------------------------------
            qt_sb = persist.tile([128, 2 * T], MMD)   # [Q01 | Q23], [d(2 heads), t]
            kt_sb = persist.tile([128, 2 * T], MMD)
            v_sb = persist.tile([128, NTB * 260], MMD)  # per j-block: 4x(64 v + 1 one)
            ctx0 = persist.tile([128, T], MMD)        # heads 0,1 ctxT
            ctx1 = persist.tile([128, T], MMD)        # heads 2,3 ctxT

            # ones columns of v_sb (denominator trick)
            nc.vector.memset(
                v_sb.rearrange("p (tb h d) -> p tb h d", tb=NTB, h=4)[:, :, :, 64:65],
                1.0,
            )

            # ---- work-unit emitters ----------------------------------------
            def emit_proj_mt(tci, mt):
                # q/k M-tiles: 0=Q(h0,h1) 1=Q(h2,h3) 2=K(h0,h1) 3=K(h2,h3)
                tsl = slice(tci * TCH, (tci + 1) * TCH)
                pq = ppj.tile([128, TCH], F32, tag="pj", name=f"pq{tci}_{mt}")
                for cb in range(CB):
                    nc.tensor.matmul(
                        pq,
                        lhsT=wqk_sb[:, cb * 512 + mt * 128 : cb * 512 + (mt + 1) * 128],
                        rhs=xt_slice(tci, cb),
                        start=(cb == 0),
                        stop=(cb == CB - 1),
                    )
                # rope: out = (pq+b)*cos + swap(pq+b)*sin  (sin sign-folded)
                m1 = work.tile([128, TCH], MMD, tag="m1", bufs=2, name=f"m1_{tci}_{mt}")
                nc.vector.scalar_tensor_tensor(
                    out=m1, in0=pq, scalar=bqk_sb[:, mt : mt + 1],
                    in1=cos_sb[:, tsl], op0=ALU.add, op1=ALU.mult,
                )
                swp = work.tile([128, TCH], MMD, tag="swp", bufs=2, name=f"swp{tci}_{mt}")
                if QB_ACT:
                    qb = work.tile([128, TCH], MMD, tag="qb", bufs=2, name=f"qb{tci}_{mt}")
                    nc.scalar.activation(
                        qb, pq, AF.Identity, bias=bqk_sb[:, mt : mt + 1], scale=1.0
                    )
                    for dst0, src0 in ((0, 32), (32, 0), (64, 96), (96, 64)):
                        nc.vector.tensor_mul(
                            swp[dst0 : dst0 + 32, :],
                            qb[src0 : src0 + 32, :],
                            sin_sb[src0 : src0 + 32, tsl],
                        )
                else:
                    # read pq straight from psum; bias rides the STT scalar
                    for dst0, src0 in ((0, 32), (32, 0), (64, 96), (96, 64)):
                        nc.vector.scalar_tensor_tensor(
                            out=swp[dst0 : dst0 + 32, :],
                            in0=pq[src0 : src0 + 32, :],
                            scalar=bqk_sb[src0 : src0 + 32, mt : mt + 1],
                            in1=sin_sb[src0 : src0 + 32, tsl],
                            op0=ALU.add, op1=ALU.mult,
                        )
                dest = qt_sb if mt < 2 else kt_sb
                dcol = (mt % 2) * T + tci * TCH
                nc.vector.tensor_add(dest[:, dcol : dcol + TCH], m1, swp)

            def emit_v_unit(tci, u):
                # v projection, natural layout [t, d]; 2 t-blocks per psum tile
                pv = ppj.tile([128, TCH], F32, tag="pj", name=f"pv{tci}_{u}")
                for q in range(2):
                    tb2 = u * 2 + q
                    for cb in range(CB):
                        nc.tensor.matmul(
                            pv[:, q * 256 : (q + 1) * 256],
                            lhsT=xt_slice(tci, cb, tb2 * 128, (tb2 + 1) * 128),
                            rhs=wv_sb[:, cb * 256 : (cb + 1) * 256],
                            start=(cb == 0),
                            stop=(cb == CB - 1),
                        )
                for q in range(2):
                    tb = tci * 4 + u * 2 + q
                    nc.vector.tensor_copy(
                        v_sb[:, tb * 260 : tb * 260 + 260].rearrange(
                            "p (h d) -> p h d", h=4
                        )[:, :, 0:64],
                        pv[:, q * 256 : (q + 1) * 256].rearrange(
                            "p (h d) -> p h d", h=4
                        ),
                    )

            def emit_outproj_unit(tci, tb2, tail=False):
                # output rows [tb*128, +128) x all 1024 cols (2 psum tiles)
                tb = tci * 4 + tb2
                ysb = work.tile([128, 1024], MMD, tag="ysb", bufs=2, name=f"ysb{tb}")
                for ncol in range(2):
                    yp = ppj.tile([128, TCH], F32, tag="pj", name=f"yp{tb}_{ncol}")
                    for cbp in range(2):
                        ctx_t = ctx0 if cbp == 0 else ctx1
                        nc.tensor.matmul(
                            yp,
                            lhsT=ctx_t[:, tb * 128 : (tb + 1) * 128],
                            rhs=wproj_sb[:, cbp * C + ncol * 512 : cbp * C + (ncol + 1) * 512],
                            start=(cbp == 0),
                            stop=(cbp == 1),
                        )
                    if tail:
                        # scalar engine is exp-free at the tail
                        nc.scalar.activation(
                            ysb[:, ncol * 512 : (ncol + 1) * 512], yp,
                            AF.Identity, scale=1.0,
                        )
                    else:
                        nc.vector.tensor_copy(ysb[:, ncol * 512 : (ncol + 1) * 512], yp)
                nc.sync.dma_start(out=yout[tb * 128 : (tb + 1) * 128, :], in_=ysb)

            def make_norm(pair, ici, ctxps, ctx_p):
                # ctx[d,i] = ctxps[d,i] / ctxps[64,i]: evict both heads'
                # denominator rows into one [2,512] tile, joint reciprocal
                # (same DVE wall as one row), broadcast, one multiply each.
                isl = slice(ici * TCH, (ici + 1) * TCH)

                def emit_norm():
                    for hh in range(2):
                        dn = work.tile([1, TCH], F32, tag="dnrow", bufs=2,
                                       name=f"dn{pair}_{ici}_{hh}")
                        nc.vector.tensor_copy(dn, ctxps[hh][64:65, :])
                        bcast = work.tile([64, TCH], F32, tag="bcast", bufs=2,
                                          name=f"bc{pair}_{ici}_{hh}")
                        nc.gpsimd.partition_broadcast(bcast, dn)
                        if KDIV:
                            nc.vector.tensor_tensor(
                                out=ctx_p[hh * 64 : (hh + 1) * 64, isl],
                                in0=ctxps[hh][0:64, :],
                                in1=bcast,
                                op=ALU.divide,
                            )
                        else:
                            rc = work.tile([64, TCH], F32, tag="recip", bufs=2,
                                           name=f"rc{pair}_{ici}_{hh}")
                            nc.vector.reciprocal_approx_fast(out=rc, in_=bcast)
                            nc.vector.tensor_mul(
                                ctx_p[hh * 64 : (hh + 1) * 64, isl],
                                ctxps[hh][0:64, :],
                                rc,
                            )

                return emit_norm

            # ---- attention: one flat (ici, pair, jb) stream ----------------
            # QK runs one slot AHEAD of exp so the scalar engine (the
            # conveyor) never waits on PE queue position; AV trails its exp
            # by AV_DEPTH slots; fillers are injected per slot; normalize is
            # scheduled a couple of slots after its pair's AVs flush.
            # chunk 0 projection up front; all q/k M-tiles before the v
            # units (v waits on the later-arriving wv DMA and must not
            # head-of-line-block the qk projection matmuls)
            for emitter in [
                lambda: emit_proj_mt(0, 0), lambda: emit_proj_mt(0, 2),
                lambda: emit_proj_mt(0, 1), lambda: emit_proj_mt(0, 3),
                lambda: emit_v_unit(0, 0), lambda: emit_v_unit(0, 1),
            ]:
                emitter()

            slots = [
                (ici, pair, jb)
                for ici in range(NCH)
                for pair in range(2)
                for jb in range(4 * (ici + 1))
            ]

            def proj_units(t):
                return [
                    (lambda mt=mt: emit_proj_mt(t, mt)) for mt in range(4)
                ] + [
                    (lambda u=u: emit_v_unit(t, u)) for u in range(2)
                ]

            def outproj_units(t):
                return [(lambda tb2=tb2: emit_outproj_unit(t, tb2)) for tb2 in range(4)]

            filler_plan = {
                0: proj_units(1),
                1: proj_units(2) + outproj_units(0),
                2: proj_units(3) + outproj_units(1),
                3: outproj_units(2),
            }

            ctxps_of = {}
            pending_exp = []     # (ici, pair, jb, st) awaiting exp emission
            pending_av = {}      # (ici, pair) -> [(at, jb), ...]
            scheduled = []       # (slot_idx, closure) delayed work

            def get_ctxps(ici, pair):
                key = (ici, pair)
                if key not in ctxps_of:
                    ctxps_of[key] = [
                        pctx.tile([65, TCH], F32, tag="ctx", bufs=2,
                                  name=f"ctxp{pair}_{ici}_{hh}")
                        for hh in range(2)
                    ]
                return ctxps_of[key]

            def emit_qk(ici, pair, jb):
                q = jb - 4 * ici
                c0 = q * 128 if (q > 0 and TRIM) else 0
                qt_p = qt_sb[:, pair * T : (pair + 1) * T]
                kt_p = kt_sb[:, pair * T : (pair + 1) * T]
                st = pst.tile([128, 1024], F32, tag="st", name=f"st{pair}_{ici}_{jb}")
                for hh in range(2):
                    nc.tensor.matmul(
                        st[:, hh * 512 + c0 : (hh + 1) * 512],
                        lhsT=kt_p[hh * 64 : (hh + 1) * 64, jb * 128 : (jb + 1) * 128],
                        rhs=qt_p[hh * 64 : (hh + 1) * 64, ici * TCH + c0 : (ici + 1) * TCH],
                        start=True,
                        stop=True,
                    )
                return st

            def emit_exp(ici, pair, jb, st):
                q = jb - 4 * ici
                c0 = q * 128 if (q > 0 and TRIM) else 0
                at = attnp.tile([128, 1024], MMD, tag="attn", bufs=6,
                                name=f"at{pair}_{ici}_{jb}")
                if TRIM and q >= 1:
                    at2 = at.rearrange("p (h i) -> p h i", h=2)
                    st2 = st.rearrange("p (h i) -> p h i", h=2)
                    nc.scalar.activation(
                        at2[:, :, c0:], st2[:, :, c0:], AF.Exp, scale=0.125
                    )
                else:
                    nc.scalar.activation(at, st, AF.Exp, scale=0.125)
                if q >= 0:
                    if TRIM:
                        tri = at.rearrange("p (h i) -> p h i", h=2)[
                            :, :, q * 128 : (q + 1) * 128
                        ]
                        nc.gpsimd.affine_select(
                            out=tri, in_=tri, compare_op=ALU.is_ge, fill=0.0,
                            base=0, channel_multiplier=-1, pattern=[[0, 2], [1, 128]],
                        )
                    else:
                        for hh in range(2):
                            half_v = at[:, hh * 512 : (hh + 1) * 512]
                            nc.gpsimd.affine_select(
                                out=half_v, in_=half_v, compare_op=ALU.is_ge,
                                fill=0.0, base=ici * TCH - jb * 128,
                                channel_multiplier=-1, pattern=[[1, 512]],
                            )
                pending_av.setdefault((ici, pair), []).append((at, jb))

            def emit_av(ici, pair, at, jb):
                njb = 4 * (ici + 1)
                q = jb - 4 * ici
                c0 = q * 128 if (q > 0 and TRIM) else 0
                ctxps = get_ctxps(ici, pair)
                for hh in range(2):
                    h_loc = pair * 2 + hh
                    nc.tensor.matmul(
                        ctxps[hh][:, c0:],
                        lhsT=v_sb[:, jb * 260 + h_loc * 65 : jb * 260 + (h_loc + 1) * 65],
                        rhs=at[:, hh * 512 + c0 : (hh + 1) * 512],
                        start=(jb == 0),
                        stop=(jb == njb - 1),
                    )

            def drain_exp(upto_len):
                while len(pending_exp) > upto_len:
                    e_ici, e_pair, e_jb, e_st = pending_exp.pop(0)
                    emit_exp(e_ici, e_pair, e_jb, e_st)
                    # AV trails exp by AV_DEPTH within its pair
                    key = (e_ici, e_pair)
                    if len(pending_av.get(key, [])) > AV_DEPTH:
                        emit_av(e_ici, e_pair, *pending_av[key].pop(0))
                    if e_jb == 4 * (e_ici + 1) - 1:
                        # pair complete: flush its AVs, schedule normalize
                        for at_jb in pending_av.pop(key, []):
                            emit_av(e_ici, e_pair, *at_jb)
                        ctx_p = ctx0 if e_pair == 0 else ctx1
                        norm = make_norm(e_pair, e_ici, ctxps_of.pop(key), ctx_p)
                        scheduled.append([2, norm])

            fillers = []
            cur_ici = -1
            for ici, pair, jb in slots:
                if ici != cur_ici:
                    fillers.extend(filler_plan[ici])
                    cur_ici = ici
                st = emit_qk(ici, pair, jb)
                pending_exp.append((ici, pair, jb, st))
                drain_exp(1)   # exp trails QK by one slot
                # delayed work (normalize) due this slot
                for item in scheduled:
                    item[0] -= 1
                for item in [it for it in scheduled if it[0] <= 0]:
                    scheduled.remove(item)
                    item[1]()
                # fillers: spread over remaining slots of this chunk
                slots_left = sum(
                    1 for (i2, p2, j2) in slots
                    if (i2, p2, j2) > (ici, pair, jb) and i2 == ici
                ) + 1
                if fillers:
                    n = -(-len(fillers) // slots_left)
                    for _ in range(min(n, len(fillers))):
                        fillers.pop(0)()
            drain_exp(0)
            for item in scheduled:
                item[1]()
            for f in fillers:
                f()
            for tb2 in range(4):
                emit_outproj_unit(NCH - 1, tb2, tail=True)

    nc.compile()
    return nc


def _prep_core_inputs(x, cos, sin, w_attn, b_attn, w_proj):
    """Build the 8 per-core input maps (host-side shard/reorder)."""
    import ml_dtypes
    mmnp = ml_dtypes.bfloat16
    x = np.asarray(x, dtype=np.float32)
    cos = np.asarray(cos, dtype=np.float32).reshape(T, HALF)
    sin = np.asarray(sin, dtype=np.float32).reshape(T, HALF)
    w_attn = np.asarray(w_attn, dtype=np.float32)
    b_attn = np.asarray(b_attn, dtype=np.float32)
    w_proj = np.asarray(w_proj, dtype=np.float32)

    cosT = np.ascontiguousarray(cos.T)               # [32, T]
    sinT = np.ascontiguousarray(sin.T)
    cosrep = np.tile(cosT, (2, 1)).astype(mmnp)      # [64, T]
    sin_sw = np.concatenate([sinT, -sinT], axis=0).astype(mmnp)  # [64, T]

    # xt rows: [p, tci, cb, t'] — identical to the device SBUF layout
    xts = []
    for b in range(B):
        xb = np.ascontiguousarray(x[b].T).astype(mmnp)      # [C, T]
        x4 = xb.reshape(CB, 128, NCH, TCH).transpose(1, 2, 0, 3)  # [p,tci,cb,t']
        xts.append(np.ascontiguousarray(x4).reshape(128, NCH * CB * TCH))

    def pack_rows(wmat, inner):
        # [CB*128, inner] -> [128, CB*inner] partition-major contiguous rows
        return np.ascontiguousarray(
            wmat.reshape(CB, 128, inner).transpose(1, 0, 2)
        ).reshape(128, CB * inner)

    in_maps = []
    for core in range(NCORES):
        b = core // 4
        g = core % 4
        heads = [4 * g + i for i in range(NH)]
        # q/k column blocks: M-tiles [Q(h0,h1), Q(h2,h3), K(h0,h1), K(h2,h3)]
        qcols, bq = [], []
        for mt, (base, hs) in enumerate(
            [(0, heads[0:2]), (0, heads[2:4]), (C, heads[0:2]), (C, heads[2:4])]
        ):
            cols = np.concatenate([np.arange(base + h * HS, base + (h + 1) * HS) for h in hs])
            qcols.append(cols)
            bq.append(b_attn[cols])
        wqk_c = pack_rows(w_attn[:, np.concatenate(qcols)].astype(mmnp), 512)
        bqk_c = np.stack(bq).T                                          # [128, 4]
        vcols = np.concatenate(
            [np.arange(2 * C + h * HS, 2 * C + (h + 1) * HS) for h in heads]
        )
        wv_c = pack_rows(w_attn[:, vcols].astype(mmnp), 256)
        wproj_c = np.ascontiguousarray(
            w_proj[g * 256 : (g + 1) * 256, :].astype(mmnp)
            .reshape(2, 128, C).transpose(1, 0, 2)
        ).reshape(128, 2 * C)
        in_maps.append(
            {
                "xt": xts[b],
                "wqk": wqk_c,
                "wv": wv_c,
                "wproj": wproj_c,
                "bqk": np.ascontiguousarray(bqk_c),
                "cosrep": np.ascontiguousarray(cosrep),
                "sinsw": np.ascontiguousarray(sin_sw),
            }
        )
    return in_maps


def kernel(x, cos, sin, w_attn, b_attn, w_proj, b_proj, _want_trace=False):
    if "nc" not in _CACHED:
        _CACHED["nc"] = _build_nc()
    nc = _CACHED["nc"]
    in_maps = _prep_core_inputs(x, cos, sin, w_attn, b_attn, w_proj)
    res = run_bass_kernel_spmd(
        nc, in_maps, core_ids=list(range(NCORES)), trace=_want_trace
    )
    _CACHED["last_result"] = res
    b_proj = np.asarray(b_proj, dtype=np.float32)
    # v-bias folds out of attention (softmax rows sum to 1): it contributes a
    # constant b_v @ w_proj to every output row, added here with b_proj.
    bv = np.asarray(b_attn, dtype=np.float32)[2 * C : 3 * C]
    bias_full = b_proj + bv @ np.asarray(w_proj, dtype=np.float32)
    out = np.empty((B, T, C), np.float32)
    for b in range(B):
        acc = res.results[b * 4]["yout"].astype(np.float32)
        for g in range(1, 4):
            acc += res.results[b * 4 + g]["yout"].astype(np.float32)
        out[b] = acc + bias_full[None, :]
    return out
